# revision 12
# baseline (speedup 1.0000x reference)
"""Deformable conv (3x3, with offset-predicting conv) for Trainium2, 8 cores.

Sharding: pure data parallel. Core k handles sample b = k//2, output row block
(k%2)*48 .. +48 (48 rows x 96 cols = 4608 pixels).

v2 pipeline per core:
  A. offset conv (3x3, C=256 -> 18) as 18 PE matmuls per 4-row chunk
  B. PE-transpose offsets into pixel-partition layout doff [128px, 36t, 18]
  C. (per 3-stage group) DVE weight chain -> factored bilinear weights
     wa/wb/wy0/wy1 [128, 36, 9] f32, plus gather index idxf (f32, exact ints)
  D. (per group) fold idxf into the SWDGE wrapped-16-partition layout via 8
     identity-slice PE matmuls + strided int16 copies; replicate x8 with 3
     log-doubling DMAs (replaces v1's 129 tiny strided DMAs ~800us)
  E. per (stage of 512px, tap): ONE dma_gather on a host-built row-pair token
     table (2KB descriptor = all 4 bilinear corners of a pixel); factored
     combine split across DVE/ACT/Pool; ONE XBAR dma transpose per tap to
     channel layout (replaces 8 PE transposes + 8 PSUM copies); PE matmul
     accumulating over (c, tap) into PSUM [o, px]; bias + store.
"""

import dataclasses

import numpy as np

import concourse.bacc as bacc
import concourse.bass as bass
import concourse.mybir as mybir
import concourse.tile as tile
from concourse import bass_utils, masks
from concourse.mybir import ActivationFunctionType as Act
from concourse.mybir import AluOpType as Op

P = 128
B, C, H, W, O = 4, 256, 96, 96, 256
K = 3
K2 = 9
NCORES = 8
ROWS = 48                      # output rows per core
NPIX = ROWS * W                # 4608
NTILE = NPIX // P              # 36 pixel tiles of 128
NSTAGE = 9                     # stages of 512 px
TPS = 4                        # pixel tiles per stage
SPX = TPS * P                  # 512
NGRP = 3                       # stage groups for prefix/stage-E overlap
SPG = NSTAGE // NGRP           # stages per group = 3
TPG = SPG * TPS                # tiles per group = 12
PADH, PADW = ROWS + 2, W + 2   # 50, 98
NPTOK = 97 * W + 2             # row-pair tokens (j in 0..96) + 2 zero pads
CONV_ROWS_PER_CHUNK = 4        # offset-conv N chunk = 4 rows = 384 cols
NCHUNK = ROWS // CONV_ROWS_PER_CHUNK  # 12
BF = mybir.dt.bfloat16
F32 = mybir.dt.float32
I16 = mybir.dt.int16

_BUILT = {}


def _emit(tc, nc, io):
    pt, xc, wofl, boff, wdcl, bdc, pyb, pxb, rep, out = io

    with (
        tc.tile_pool(name="const", bufs=1) as cpool,
        tc.tile_pool(name="sbig", bufs=1) as spool,
    ):
        ident_f = cpool.tile([P, P], F32, tag="idf", name="idf")
        masks.make_identity(nc, ident_f[:])

        # ---- persistent SBUF buffers ----
        xc_sb = spool.tile([P, 2, PADH * PADW], BF, tag="xc", name="xc")
        wofl_sb = spool.tile([P, 2, K2, 18], BF, tag="wofl", name="wofl")
        wdcl_sb = spool.tile([P, K2, 2, 2, P], BF, tag="wdcl", name="wdcl")
        boff_sb = spool.tile([18, 1], F32, tag="boff", name="boff")
        bdc_sb = spool.tile([P, 2], F32, tag="bdc", name="bdc")
        pyb_sb = spool.tile([P, NTILE, K2], F32, tag="pyb", name="pyb")
        pxb_sb = spool.tile([P, NTILE, K2], F32, tag="pxb", name="pxb")
        rep_sb = spool.tile([P, 8, P], F32, tag="rep", name="rep")
        off_sb = spool.tile([18, NPIX], F32, tag="off", name="off")
        doff = spool.tile([P, NTILE, 18], F32, tag="doff", name="doff")
        # factored bilinear weights (per px, tap)
        wa_sb = spool.tile([P, NTILE, K2], F32, tag="wa", name="wa")
        wb_sb = spool.tile([P, NTILE, K2], F32, tag="wb", name="wb")
        wy0_sb = spool.tile([P, NTILE, K2], F32, tag="wy0", name="wy0")
        wy1_sb = spool.tile([P, NTILE, K2], F32, tag="wy1", name="wy1")
        # wrapped gather indices [16 parts x8 rep, (s, tap, t, k)]
        cidxw = spool.tile([P, NSTAGE, K2, TPS, 8], I16, tag="cidxw", name="cidxw")

        nc.sync.dma_start(xc_sb[:], xc)
        nc.sync.dma_start(wofl_sb[:], wofl)
        nc.sync.dma_start(wdcl_sb[:], wdcl)
        nc.sync.dma_start(boff_sb[:], boff)
        nc.sync.dma_start(bdc_sb[:], bdc)
        nc.sync.dma_start(pyb_sb[:], pyb)
        nc.sync.dma_start(pxb_sb[:], pxb)
        nc.sync.dma_start(rep_sb[:], rep)

        with (
            tc.tile_pool(name="psA", bufs=2, space="PSUM") as psa,
            tc.tile_pool(name="psB", bufs=1, space="PSUM") as psb,
            tc.tile_pool(name="psF", bufs=1, space="PSUM") as psf_pool,
            tc.tile_pool(name="scr", bufs=2) as scr,
            tc.tile_pool(name="gpool", bufs=5) as gpool,
            tc.tile_pool(name="mpool", bufs=4) as mpool,
            tc.tile_pool(name="vapool", bufs=3) as vapool,
            tc.tile_pool(name="rpool", bufs=3) as rpool,
            tc.tile_pool(name="opool", bufs=2) as opool,
            tc.tile_pool(name="psO", bufs=2, space="PSUM") as pso,
        ):
            # ---- A: offset conv ----
            for ch_i in range(NCHUNK):
                ncols = CONV_ROWS_PER_CHUNK * W  # 384
                ps = psa.tile([18, ncols], F32, tag="psoff", name="psoff")
                n_mm = 2 * K2
                mm = 0
                xcf = xc_sb[:]
                for chalf in range(2):
                    for tap in range(K2):
                        ti, tj = tap // K, tap % K
                        rhs = dataclasses.replace(
                            xcf,
                            ap=[
                                [xcf.ap[0][0], P],
                                [PADW, CONV_ROWS_PER_CHUNK],
                                [1, W],
                            ],
                            offset=xcf.offset
                            + chalf * (PADH * PADW)
                            + ((ch_i * CONV_ROWS_PER_CHUNK + ti) * PADW + tj),
                        )
                        nc.tensor.matmul(
                            ps[:],
                            wofl_sb[:, chalf, tap],
                            rhs,
                            start=(mm == 0),
                            stop=(mm == n_mm - 1),
                        )
                        mm += 1
                nc.scalar.activation(
                    off_sb[:, ch_i * ncols : (ch_i + 1) * ncols],
                    ps[:],
                    Act.Identity,
                    bias=boff_sb[:],
                )

            # ---- B: transpose offsets to pixel layout ----
            for t in range(NTILE):
                pt_ps = psb.tile([P, 18], F32, tag="pofft", name="pofft")
                nc.tensor.transpose(
                    pt_ps[:], off_sb[:, t * P : (t + 1) * P], ident_f[:18, :18]
                )
                nc.scalar.copy(doff[:, t, :], pt_ps[:])

            # ---- C+D per group, then E per stage ----
            MAGIC = 8388608.0  # 2^23

            def chain(g):
                # weight + index math on [P, TPG, K2] slices (group g)
                sl = slice(g * TPG, (g + 1) * TPG)
                sh = [P, TPG, K2]

                def tmp(tag):
                    return scr.tile(sh, F32, tag=tag, name=tag)

                dy = doff[:, sl, 0:18:2]
                dx = doff[:, sl, 1:18:2]
                # pyb/pxb already carry +16 (host); coords strictly positive
                py = tmp("py")
                px = tmp("px")
                nc.vector.tensor_tensor(py[:], pyb_sb[:, sl], dy, Op.add)
                nc.vector.tensor_tensor(px[:], pxb_sb[:, sl], dx, Op.add)
                y0 = tmp("y0")
                x0 = tmp("x0")
                nc.vector.tensor_scalar(y0[:], py[:], -0.4999999, None, Op.add)
                nc.vector.tensor_scalar(y0[:], y0[:], MAGIC, -MAGIC, Op.add, Op.add)
                nc.vector.tensor_scalar(x0[:], px[:], -0.4999999, None, Op.add)
                nc.vector.tensor_scalar(x0[:], x0[:], MAGIC, -MAGIC, Op.add, Op.add)
                ly = tmp("ly")
                lx = tmp("lx")
                nc.vector.tensor_tensor(ly[:], py[:], y0[:], Op.subtract)
                nc.vector.tensor_tensor(lx[:], px[:], x0[:], Op.subtract)

                ta_ = tmp("ta")
                tb_ = tmp("tb")
                tc_ = tmp("tc")
                # y: wy0 = (1-ly)*[16<=y0<=111], wy1 = ly*[15<=y0<=110]
                nc.vector.tensor_scalar(ta_[:], y0[:], 16.0, None, Op.is_ge)
                nc.vector.tensor_scalar(tb_[:], y0[:], 111.0, None, Op.is_le)
                vy0 = tmp("vy0")
                nc.vector.tensor_tensor(vy0[:], ta_[:], tb_[:], Op.mult)
                nc.vector.tensor_scalar(ta_[:], y0[:], 15.0, None, Op.is_ge)
                nc.vector.tensor_scalar(tb_[:], y0[:], 110.0, None, Op.is_le)
                vy1 = tmp("vy1")
                nc.vector.tensor_tensor(vy1[:], ta_[:], tb_[:], Op.mult)
                nc.vector.tensor_scalar(tc_[:], ly[:], -1.0, 1.0, Op.mult, Op.add)
                nc.vector.tensor_tensor(wy0_sb[:, sl], tc_[:], vy0[:], Op.mult)
                nc.vector.tensor_tensor(wy1_sb[:, sl], ly[:], vy1[:], Op.mult)

                # x pair weights: wa = (1-lx)*[16<=x0<=111] + lx*[x0==15]
                #                 wb = lx*[16<=x0<=110]
                nc.vector.tensor_scalar(ta_[:], x0[:], 16.0, None, Op.is_ge)
                nc.vector.tensor_scalar(tb_[:], x0[:], 111.0, None, Op.is_le)
                vx = tmp("vx")
                nc.vector.tensor_tensor(vx[:], ta_[:], tb_[:], Op.mult)
                nc.vector.tensor_scalar(tb_[:], x0[:], 110.0, None, Op.is_le)
                vxb = tmp("vxb")
                nc.vector.tensor_tensor(vxb[:], ta_[:], tb_[:], Op.mult)
                td_ = tmp("td")
                nc.vector.tensor_scalar(td_[:], x0[:], 15.0, None, Op.is_equal)
                nc.vector.tensor_scalar(tc_[:], lx[:], -1.0, 1.0, Op.mult, Op.add)
                nc.vector.tensor_tensor(tc_[:], tc_[:], vx[:], Op.mult)
                nc.vector.tensor_tensor(td_[:], lx[:], td_[:], Op.mult)
                nc.vector.tensor_tensor(wa_sb[:, sl], tc_[:], td_[:], Op.add)
                nc.vector.tensor_tensor(wb_sb[:, sl], lx[:], vxb[:], Op.mult)

                # gather index (f32, exact ints): pair-token j=clip(y0r,-1,95)+1,
                # xb=clip(x0r,0,95); idx = j*96+xb. Shifted: jsh=clip(y0,15,111),
                # xbsh=clip(x0,16,111), idx = jsh*96 + xbsh - 1456.
                jsh = tmp("jsh")
                xbsh = tmp("xbsh")
                nc.vector.tensor_scalar(jsh[:], y0[:], 15.0, 111.0, Op.max, Op.min)
                nc.vector.tensor_scalar(xbsh[:], x0[:], 16.0, 111.0, Op.max, Op.min)
                idxf = tmp("idxf")
                nc.vector.scalar_tensor_tensor(
                    idxf[:], jsh[:], 96.0, xbsh[:], Op.mult, Op.add
                )
                nc.vector.tensor_scalar(idxf[:], idxf[:], -1456.0, None, Op.add)

                # fold to wrapped layout: value for partition 16k+r lands on
                # ALL partitions 16g+r (rep_sb[:, k] = replicated ident slice),
                # so no replication DMAs are needed on the index path.
                for k in range(8):
                    psf = psf_pool.tile([P, TPG * K2], F32, tag="psf", name="psf")
                    nc.tensor.matmul(
                        psf[:],
                        rep_sb[:, k],
                        idxf[:],
                        start=True,
                        stop=True,
                    )
                    # src iter (s_loc, t_loc, tap) -> dst strides (288, 8, 32)
                    cw = cidxw[:]
                    dst = dataclasses.replace(
                        cw,
                        ap=[
                            [cw.ap[0][0], P],
                            [K2 * TPS * 8, SPG],
                            [8, TPS],
                            [TPS * 8, K2],
                        ],
                        offset=cw.offset + g * SPG * K2 * TPS * 8 + k,
                    )
                    src = dataclasses.replace(
                        psf[:],
                        ap=[
                            [psf[:].ap[0][0], P],
                            [TPS * K2, SPG],
                            [K2, TPS],
                            [1, K2],
                        ],
                    )
                    nc.vector.tensor_copy(dst, src)

            # overlapped-window view of pair-token table: [tok, 1024] stride 512
            pt_win = dataclasses.replace(
                pt, ap=[[2 * C, NPTOK - 1], [1, 4 * C]], offset=0
            )

            def stage(s):
                po = [
                    pso.tile([P, SPX], F32, tag=f"po{oh}", name=f"po{oh}")
                    for oh in range(2)
                ]
                for tap in range(K2):
                    gt = gpool.tile([P, TPS, 4 * C], BF, tag="g", name="g")
                    nc.gpsimd.dma_gather(
                        gt[:],
                        pt_win,
                        cidxw[:, s, tap],
                        SPX,
                        SPX,
                        elem_size=4 * C,
                        elem_step=2 * C,
                    )
                    va = vapool.tile([P, TPS, 2 * P], BF, tag="va", name="va")
                    for t in range(TPS):
                        st_g = s * TPS + t
                        wa_c = wa_sb[:, st_g, tap : tap + 1]
                        wb_c = wb_sb[:, st_g, tap : tap + 1]
                        wy0_c = wy0_sb[:, st_g, tap : tap + 1]
                        wy1_c = wy1_sb[:, st_g, tap : tap + 1]
                        m_ = mpool.tile([P, 2 * C], BF, tag="m", name="m")
                        s_ = mpool.tile([P, 2 * C], BF, tag="s", name="s")
                        h0 = mpool.tile([P, C], BF, tag="h0", name="h0")
                        h1 = mpool.tile([P, C], BF, tag="h1", name="h1")
                        nc.vector.tensor_scalar(
                            m_[:], gt[:, t, 0 : 2 * C], wa_c, None, Op.mult
                        )
                        nc.vector.scalar_tensor_tensor(
                            s_[:], gt[:, t, 2 * C : 4 * C], wb_c, m_[:],
                            Op.mult, Op.add,
                        )
                        nc.scalar.activation(
                            h0[:], s_[:, 0:C], Act.Identity, scale=wy0_c
                        )
                        nc.scalar.activation(
                            h1[:], s_[:, C : 2 * C], Act.Identity, scale=wy1_c
                        )
                        nc.vector.tensor_tensor(
                            va[:, t, :], h0[:], h1[:], Op.add
                        )
                    # XBAR transpose: [128px, (t,chalf,c)] -> [128c, (t,chalf), px]
                    rst = rpool.tile([P, 2 * TPS, P], BF, tag="rst", name="rst")
                    nc.sync.dma_start_transpose(rst[:], va[:])
                    for chalf in range(2):
                        rv = rst[:]
                        rhs = dataclasses.replace(
                            rv,
                            ap=[[rv.ap[0][0], P], [2 * P, TPS], [1, P]],
                            offset=rv.offset + chalf * P,
                        )
                        for oh in range(2):
                            nc.tensor.matmul(
                                po[oh][:],
                                wdcl_sb[:, tap, chalf, oh],
                                rhs,
                                start=(tap == 0 and chalf == 0),
                                stop=(tap == K2 - 1 and chalf == 1),
                            )
                for oh in range(2):
                    ob = opool.tile([P, SPX], F32, tag="ob", name="ob")
                    nc.scalar.activation(
                        ob[:], po[oh][:], Act.Identity, bias=bdc_sb[:, oh : oh + 1]
                    )
                    nc.sync.dma_start(out[oh, :, s * SPX : (s + 1) * SPX], ob[:])

            for g in range(NGRP):
                chain(g)
                for s in range(g * SPG, (g + 1) * SPG):
                    stage(s)


def _build():
    if "nc" in _BUILT:
        return _BUILT["nc"]
    nc = bacc.Bacc(
        "TRN2",
        target_bir_lowering=False,
        debug=False,
        enable_asserts=False,
        num_devices=NCORES,
    )
    pt = nc.dram_tensor("pt", [NPTOK, 2 * C], BF, kind="ExternalInput").ap()
    xc = nc.dram_tensor("xc", [P, 2, PADH * PADW], BF, kind="ExternalInput").ap()
    wofl = nc.dram_tensor("wofl", [P, 2, K2, 18], BF, kind="ExternalInput").ap()
    boff = nc.dram_tensor("boff", [18, 1], F32, kind="ExternalInput").ap()
    wdcl = nc.dram_tensor("wdcl", [P, K2, 2, 2, P], BF, kind="ExternalInput").ap()
    bdc = nc.dram_tensor("bdc", [P, 2], F32, kind="ExternalInput").ap()
    pyb = nc.dram_tensor("pyb", [P, NTILE, K2], F32, kind="ExternalInput").ap()
    pxb = nc.dram_tensor("pxb", [P, NTILE, K2], F32, kind="ExternalInput").ap()
    rep = nc.dram_tensor("rep", [P, 8, P], F32, kind="ExternalInput").ap()
    out = nc.dram_tensor("out", [2, P, NPIX], F32, kind="ExternalOutput").ap()
    with tile.TileContext(nc) as tc:
        _emit(tc, nc, (pt, xc, wofl, boff, wdcl, bdc, pyb, pxb, rep, out))
    nc.compile()
    _BUILT["nc"] = nc
    return nc


def _prep_core(k, x, w_off, b_off, w_dc, b_dc):
    import ml_dtypes

    bf16 = ml_dtypes.bfloat16
    b, half = k // 2, k % 2
    y0 = half * ROWS
    xs = x[b]  # [C,H,W] f32
    xs_t = xs.transpose(1, 2, 0)  # [H, W, C]
    # row-pair token table: token (j, x) = [row j-1, row j], zero padded
    pt = np.zeros((NPTOK, 2 * C), np.float32)
    view = pt[: 97 * W].reshape(97, W, 2 * C)
    view[1:, :, 0:C] = xs_t
    view[:H, :, C : 2 * C] = xs_t

    xc = np.zeros((C, PADH, PADW), np.float32)
    r0, r1 = max(0, y0 - 1), min(H, y0 + ROWS + 1)
    xc[:, (r0 - (y0 - 1)) : (r1 - (y0 - 1)), 1 : 1 + W] = xs[:, r0:r1, :]
    xc = xc.reshape(2, P, PADH * PADW).transpose(1, 0, 2)

    wofl = (
        w_off.reshape(2 * K2, 2, P, K2)   # [oc, chalf, c, tap]
        .transpose(2, 1, 3, 0)            # [c, chalf, tap, oc]
        .copy()
    )
    wdcl = (
        w_dc.reshape(2, P, 2, P, K2)      # [oh, o, chalf, c, tap]
        .transpose(3, 4, 2, 0, 1)         # [c, tap, chalf, oh, o]
        .copy()
    )
    bdc = b_dc.reshape(2, P).transpose(1, 0).copy()

    pp = np.arange(NPIX)
    yg = y0 + pp // W
    xg = pp % W
    ti = (np.arange(K2) // K)[None, :]
    tj = (np.arange(K2) % K)[None, :]
    # +16 shift baked in (device math keeps coords strictly positive)
    pyb = (yg[:, None] - 1 + ti + 16).astype(np.float32).reshape(NTILE, P, K2)
    pxb = (xg[:, None] - 1 + tj + 16).astype(np.float32).reshape(NTILE, P, K2)

    # replicated identity slices: rep[16k+r, k, r::16] = 1
    repm = np.zeros((P, 8, P), np.float32)
    for kk in range(8):
        for r in range(16):
            repm[16 * kk + r, kk, r::16] = 1.0

    return {
        "pt": pt.astype(bf16),
        "rep": repm,
        "xc": xc.astype(bf16),
        "wofl": wofl.astype(bf16),
        "boff": b_off.reshape(18, 1).astype(np.float32),
        "wdcl": wdcl.astype(bf16),
        "bdc": bdc.astype(np.float32),
        "pyb": pyb.transpose(1, 0, 2).copy(),
        "pxb": pxb.transpose(1, 0, 2).copy(),
    }


def kernel(x, w_off, b_off, w_dc, b_dc, _trace=False):
    nc = _build()
    x = np.asarray(x, np.float32)
    w_off = np.asarray(w_off, np.float32)
    b_off = np.asarray(b_off, np.float32)
    w_dc = np.asarray(w_dc, np.float32)
    b_dc = np.asarray(b_dc, np.float32)
    in_maps = [
        _prep_core(k, x, w_off, b_off, w_dc, b_dc) for k in range(NCORES)
    ]
    res = bass_utils.run_bass_kernel_spmd(
        nc, in_maps, core_ids=list(range(NCORES)), trace=_trace
    )
    out = np.empty((B, O, H, W), np.float32)
    for k in range(NCORES):
        b, half = k // 2, k % 2
        o = res.results[k]["out"]  # [2,128,4608]
        out[b, :, half * ROWS : (half + 1) * ROWS, :] = o.reshape(
            O, ROWS, W
        )
    if _trace:
        return out, res
    return out


# revision 13
# speedup vs baseline: 1.0717x; 1.0717x over previous
"""Deformable conv (3x3, with offset-predicting conv) for Trainium2, 8 cores.

Sharding: pure data parallel. Core k handles sample b = k//2, output row block
(k%2)*48 .. +48 (48 rows x 96 cols = 4608 pixels).

v2 pipeline per core:
  A. offset conv (3x3, C=256 -> 18) as 18 PE matmuls per 4-row chunk
  B. PE-transpose offsets into pixel-partition layout doff [128px, 36t, 18]
  C. (per 3-stage group) DVE weight chain -> factored bilinear weights
     wa/wb/wy0/wy1 [128, 36, 9] f32, plus gather index idxf (f32, exact ints)
  D. (per group) fold idxf into the SWDGE wrapped-16-partition layout via 8
     identity-slice PE matmuls + strided int16 copies; replicate x8 with 3
     log-doubling DMAs (replaces v1's 129 tiny strided DMAs ~800us)
  E. per (stage of 512px, tap): ONE dma_gather on a host-built row-pair token
     table (2KB descriptor = all 4 bilinear corners of a pixel); factored
     combine split across DVE/ACT/Pool; ONE XBAR dma transpose per tap to
     channel layout (replaces 8 PE transposes + 8 PSUM copies); PE matmul
     accumulating over (c, tap) into PSUM [o, px]; bias + store.
"""

import dataclasses

import numpy as np

import concourse.bacc as bacc
import concourse.bass as bass
import concourse.mybir as mybir
import concourse.tile as tile
from concourse import bass_utils, masks
from concourse.mybir import ActivationFunctionType as Act
from concourse.mybir import AluOpType as Op

P = 128
B, C, H, W, O = 4, 256, 96, 96, 256
K = 3
K2 = 9
NCORES = 8
ROWS = 48                      # output rows per core
NPIX = ROWS * W                # 4608
NTILE = NPIX // P              # 36 pixel tiles of 128
NSTAGE = 9                     # stages of 512 px
TPS = 4                        # pixel tiles per stage
SPX = TPS * P                  # 512
NGRP = 3                       # stage groups for prefix/stage-E overlap
SPG = NSTAGE // NGRP           # stages per group = 3
TPG = SPG * TPS                # tiles per group = 12
PADH, PADW = ROWS + 2, W + 2   # 50, 98
NPTOK = 97 * W + 2             # row-pair tokens (j in 0..96) + 2 zero pads
CONV_ROWS_PER_CHUNK = 4        # offset-conv N chunk = 4 rows = 384 cols
NCHUNK = ROWS // CONV_ROWS_PER_CHUNK  # 12
BF = mybir.dt.bfloat16
F32 = mybir.dt.float32
I16 = mybir.dt.int16

_BUILT = {}


def _emit(tc, nc, io):
    pt, xc, wofl, boff, wdcl, bdc, pyb, pxb, rep, out = io

    with (
        tc.tile_pool(name="const", bufs=1) as cpool,
        tc.tile_pool(name="sbig", bufs=1) as spool,
    ):
        ident_f = cpool.tile([P, P], F32, tag="idf", name="idf")
        masks.make_identity(nc, ident_f[:])

        # ---- persistent SBUF buffers ----
        xc_sb = spool.tile([P, 2, PADH * PADW], BF, tag="xc", name="xc")
        wofl_sb = spool.tile([P, 2, K2, 18], BF, tag="wofl", name="wofl")
        wdcl_sb = spool.tile([P, K2, 2, 2, P], BF, tag="wdcl", name="wdcl")
        boff_sb = spool.tile([18, 1], F32, tag="boff", name="boff")
        bdc_sb = spool.tile([P, 2], F32, tag="bdc", name="bdc")
        pyb_sb = spool.tile([P, NTILE, K2], F32, tag="pyb", name="pyb")
        pxb_sb = spool.tile([P, NTILE, K2], F32, tag="pxb", name="pxb")
        rep_sb = spool.tile([P, 8, P], F32, tag="rep", name="rep")
        off_sb = spool.tile([18, NPIX], F32, tag="off", name="off")
        doff = spool.tile([P, NTILE, 18], F32, tag="doff", name="doff")
        # factored bilinear weights (per px, tap)
        wa_sb = spool.tile([P, NTILE, K2], F32, tag="wa", name="wa")
        wb_sb = spool.tile([P, NTILE, K2], F32, tag="wb", name="wb")
        wy0_sb = spool.tile([P, NTILE, K2], F32, tag="wy0", name="wy0")
        wy1_sb = spool.tile([P, NTILE, K2], F32, tag="wy1", name="wy1")
        # wrapped gather indices [16 parts x8 rep, (s, tap, t, k)]
        cidxw = spool.tile([P, NSTAGE, K2, TPS, 8], I16, tag="cidxw", name="cidxw")

        nc.sync.dma_start(xc_sb[:], xc)
        nc.sync.dma_start(wofl_sb[:], wofl)
        nc.sync.dma_start(wdcl_sb[:], wdcl)
        nc.sync.dma_start(boff_sb[:], boff)
        nc.sync.dma_start(bdc_sb[:], bdc)
        nc.sync.dma_start(pyb_sb[:], pyb)
        nc.sync.dma_start(pxb_sb[:], pxb)
        nc.sync.dma_start(rep_sb[:], rep)

        with (
            tc.tile_pool(name="psA", bufs=2, space="PSUM") as psa,
            tc.tile_pool(name="psB", bufs=1, space="PSUM") as psb,
            tc.tile_pool(name="psF", bufs=1, space="PSUM") as psf_pool,
            tc.tile_pool(name="scr", bufs=2) as scr,
            tc.tile_pool(name="gpool", bufs=6) as gpool,
            tc.tile_pool(name="mpool", bufs=8) as mpool,
            tc.tile_pool(name="vapool", bufs=6) as vapool,
            tc.tile_pool(name="rpool", bufs=6) as rpool,
            tc.tile_pool(name="opool", bufs=4) as opool,
            tc.tile_pool(name="psO", bufs=2, space="PSUM") as pso,
        ):
            # ---- A: offset conv ----
            for ch_i in range(NCHUNK):
                ncols = CONV_ROWS_PER_CHUNK * W  # 384
                ps = psa.tile([18, ncols], F32, tag="psoff", name="psoff")
                n_mm = 2 * K2
                mm = 0
                xcf = xc_sb[:]
                for chalf in range(2):
                    for tap in range(K2):
                        ti, tj = tap // K, tap % K
                        rhs = dataclasses.replace(
                            xcf,
                            ap=[
                                [xcf.ap[0][0], P],
                                [PADW, CONV_ROWS_PER_CHUNK],
                                [1, W],
                            ],
                            offset=xcf.offset
                            + chalf * (PADH * PADW)
                            + ((ch_i * CONV_ROWS_PER_CHUNK + ti) * PADW + tj),
                        )
                        nc.tensor.matmul(
                            ps[:],
                            wofl_sb[:, chalf, tap],
                            rhs,
                            start=(mm == 0),
                            stop=(mm == n_mm - 1),
                        )
                        mm += 1
                nc.scalar.activation(
                    off_sb[:, ch_i * ncols : (ch_i + 1) * ncols],
                    ps[:],
                    Act.Identity,
                    bias=boff_sb[:],
                )

            # ---- B: transpose offsets to pixel layout ----
            for t in range(NTILE):
                pt_ps = psb.tile([P, 18], F32, tag="pofft", name="pofft")
                nc.tensor.transpose(
                    pt_ps[:], off_sb[:, t * P : (t + 1) * P], ident_f[:18, :18]
                )
                nc.scalar.copy(doff[:, t, :], pt_ps[:])

            # ---- C+D per group, then E per stage ----
            MAGIC = 8388608.0  # 2^23

            def chain(g):
                # weight + index math on [P, TPG, K2] slices (group g)
                sl = slice(g * TPG, (g + 1) * TPG)
                sh = [P, TPG, K2]

                def tmp(tag):
                    return scr.tile(sh, F32, tag=tag, name=tag)

                dy = doff[:, sl, 0:18:2]
                dx = doff[:, sl, 1:18:2]
                # pyb/pxb already carry +16 (host); coords strictly positive
                py = tmp("py")
                px = tmp("px")
                nc.vector.tensor_tensor(py[:], pyb_sb[:, sl], dy, Op.add)
                nc.vector.tensor_tensor(px[:], pxb_sb[:, sl], dx, Op.add)
                y0 = tmp("y0")
                x0 = tmp("x0")
                nc.vector.tensor_scalar(y0[:], py[:], -0.4999999, None, Op.add)
                nc.vector.tensor_scalar(y0[:], y0[:], MAGIC, -MAGIC, Op.add, Op.add)
                nc.vector.tensor_scalar(x0[:], px[:], -0.4999999, None, Op.add)
                nc.vector.tensor_scalar(x0[:], x0[:], MAGIC, -MAGIC, Op.add, Op.add)
                ly = tmp("ly")
                lx = tmp("lx")
                nc.vector.tensor_tensor(ly[:], py[:], y0[:], Op.subtract)
                nc.vector.tensor_tensor(lx[:], px[:], x0[:], Op.subtract)

                ta_ = tmp("ta")
                tb_ = tmp("tb")
                tc_ = tmp("tc")
                # y: wy0 = (1-ly)*[16<=y0<=111], wy1 = ly*[15<=y0<=110]
                nc.vector.tensor_scalar(ta_[:], y0[:], 16.0, None, Op.is_ge)
                nc.vector.tensor_scalar(tb_[:], y0[:], 111.0, None, Op.is_le)
                vy0 = tmp("vy0")
                nc.vector.tensor_tensor(vy0[:], ta_[:], tb_[:], Op.mult)
                nc.vector.tensor_scalar(ta_[:], y0[:], 15.0, None, Op.is_ge)
                nc.vector.tensor_scalar(tb_[:], y0[:], 110.0, None, Op.is_le)
                vy1 = tmp("vy1")
                nc.vector.tensor_tensor(vy1[:], ta_[:], tb_[:], Op.mult)
                nc.vector.tensor_scalar(tc_[:], ly[:], -1.0, 1.0, Op.mult, Op.add)
                nc.vector.tensor_tensor(wy0_sb[:, sl], tc_[:], vy0[:], Op.mult)
                nc.vector.tensor_tensor(wy1_sb[:, sl], ly[:], vy1[:], Op.mult)

                # x pair weights: wa = (1-lx)*[16<=x0<=111] + lx*[x0==15]
                #                 wb = lx*[16<=x0<=110]
                nc.vector.tensor_scalar(ta_[:], x0[:], 16.0, None, Op.is_ge)
                nc.vector.tensor_scalar(tb_[:], x0[:], 111.0, None, Op.is_le)
                vx = tmp("vx")
                nc.vector.tensor_tensor(vx[:], ta_[:], tb_[:], Op.mult)
                nc.vector.tensor_scalar(tb_[:], x0[:], 110.0, None, Op.is_le)
                vxb = tmp("vxb")
                nc.vector.tensor_tensor(vxb[:], ta_[:], tb_[:], Op.mult)
                td_ = tmp("td")
                nc.vector.tensor_scalar(td_[:], x0[:], 15.0, None, Op.is_equal)
                nc.vector.tensor_scalar(tc_[:], lx[:], -1.0, 1.0, Op.mult, Op.add)
                nc.vector.tensor_tensor(tc_[:], tc_[:], vx[:], Op.mult)
                nc.vector.tensor_tensor(td_[:], lx[:], td_[:], Op.mult)
                nc.vector.tensor_tensor(wa_sb[:, sl], tc_[:], td_[:], Op.add)
                nc.vector.tensor_tensor(wb_sb[:, sl], lx[:], vxb[:], Op.mult)

                # gather index (f32, exact ints): pair-token j=clip(y0r,-1,95)+1,
                # xb=clip(x0r,0,95); idx = j*96+xb. Shifted: jsh=clip(y0,15,111),
                # xbsh=clip(x0,16,111), idx = jsh*96 + xbsh - 1456.
                jsh = tmp("jsh")
                xbsh = tmp("xbsh")
                nc.vector.tensor_scalar(jsh[:], y0[:], 15.0, 111.0, Op.max, Op.min)
                nc.vector.tensor_scalar(xbsh[:], x0[:], 16.0, 111.0, Op.max, Op.min)
                idxf = tmp("idxf")
                nc.vector.scalar_tensor_tensor(
                    idxf[:], jsh[:], 96.0, xbsh[:], Op.mult, Op.add
                )
                nc.vector.tensor_scalar(idxf[:], idxf[:], -1456.0, None, Op.add)

                # fold to wrapped layout: value for partition 16k+r lands on
                # ALL partitions 16g+r (rep_sb[:, k] = replicated ident slice),
                # so no replication DMAs are needed on the index path.
                for k in range(8):
                    psf = psf_pool.tile([P, TPG * K2], F32, tag="psf", name="psf")
                    nc.tensor.matmul(
                        psf[:],
                        rep_sb[:, k],
                        idxf[:],
                        start=True,
                        stop=True,
                    )
                    # src iter (s_loc, t_loc, tap) -> dst strides (288, 8, 32)
                    cw = cidxw[:]
                    dst = dataclasses.replace(
                        cw,
                        ap=[
                            [cw.ap[0][0], P],
                            [K2 * TPS * 8, SPG],
                            [8, TPS],
                            [TPS * 8, K2],
                        ],
                        offset=cw.offset + g * SPG * K2 * TPS * 8 + k,
                    )
                    src = dataclasses.replace(
                        psf[:],
                        ap=[
                            [psf[:].ap[0][0], P],
                            [TPS * K2, SPG],
                            [K2, TPS],
                            [1, K2],
                        ],
                    )
                    nc.vector.tensor_copy(dst, src)

            # overlapped-window view of pair-token table: [tok, 1024] stride 512
            pt_win = dataclasses.replace(
                pt, ap=[[2 * C, NPTOK - 1], [1, 4 * C]], offset=0
            )

            def stage(s):
                po = [
                    pso.tile([P, SPX], F32, tag=f"po{oh}", name=f"po{oh}")
                    for oh in range(2)
                ]
                for tap in range(K2):
                    gt = gpool.tile([P, TPS, 4 * C], BF, tag="g", name="g")
                    nc.gpsimd.dma_gather(
                        gt[:],
                        pt_win,
                        cidxw[:, s, tap],
                        SPX,
                        SPX,
                        elem_size=4 * C,
                        elem_step=2 * C,
                    )
                    va = vapool.tile([P, TPS, 2 * P], BF, tag="va", name="va")
                    for t in range(TPS):
                        st_g = s * TPS + t
                        wa_c = wa_sb[:, st_g, tap : tap + 1]
                        wb_c = wb_sb[:, st_g, tap : tap + 1]
                        wy0_c = wy0_sb[:, st_g, tap : tap + 1]
                        wy1_c = wy1_sb[:, st_g, tap : tap + 1]
                        m_ = mpool.tile([P, 2 * C], BF, tag="m", name="m")
                        s_ = mpool.tile([P, 2 * C], BF, tag="s", name="s")
                        h0 = mpool.tile([P, C], BF, tag="h0", name="h0")
                        h1 = mpool.tile([P, C], BF, tag="h1", name="h1")
                        nc.vector.tensor_scalar(
                            m_[:], gt[:, t, 0 : 2 * C], wa_c, None, Op.mult
                        )
                        nc.vector.scalar_tensor_tensor(
                            s_[:], gt[:, t, 2 * C : 4 * C], wb_c, m_[:],
                            Op.mult, Op.add,
                        )
                        nc.scalar.activation(
                            h0[:], s_[:, 0:C], Act.Identity, scale=wy0_c
                        )
                        nc.scalar.activation(
                            h1[:], s_[:, C : 2 * C], Act.Identity, scale=wy1_c
                        )
                        nc.vector.tensor_tensor(
                            va[:, t, :], h0[:], h1[:], Op.add
                        )
                    # XBAR transpose: [128px, (t,chalf,c)] -> [128c, (t,chalf), px]
                    rst = rpool.tile([P, 2 * TPS, P], BF, tag="rst", name="rst")
                    nc.sync.dma_start_transpose(rst[:], va[:])
                    for chalf in range(2):
                        rv = rst[:]
                        rhs = dataclasses.replace(
                            rv,
                            ap=[[rv.ap[0][0], P], [2 * P, TPS], [1, P]],
                            offset=rv.offset + chalf * P,
                        )
                        for oh in range(2):
                            nc.tensor.matmul(
                                po[oh][:],
                                wdcl_sb[:, tap, chalf, oh],
                                rhs,
                                start=(tap == 0 and chalf == 0),
                                stop=(tap == K2 - 1 and chalf == 1),
                            )
                for oh in range(2):
                    ob = opool.tile([P, SPX], F32, tag="ob", name="ob")
                    nc.scalar.activation(
                        ob[:], po[oh][:], Act.Identity, bias=bdc_sb[:, oh : oh + 1]
                    )
                    nc.sync.dma_start(out[oh, :, s * SPX : (s + 1) * SPX], ob[:])

            for g in range(NGRP):
                chain(g)
                for s in range(g * SPG, (g + 1) * SPG):
                    stage(s)


def _build():
    if "nc" in _BUILT:
        return _BUILT["nc"]
    nc = bacc.Bacc(
        "TRN2",
        target_bir_lowering=False,
        debug=False,
        enable_asserts=False,
        num_devices=NCORES,
    )
    pt = nc.dram_tensor("pt", [NPTOK, 2 * C], BF, kind="ExternalInput").ap()
    xc = nc.dram_tensor("xc", [P, 2, PADH * PADW], BF, kind="ExternalInput").ap()
    wofl = nc.dram_tensor("wofl", [P, 2, K2, 18], BF, kind="ExternalInput").ap()
    boff = nc.dram_tensor("boff", [18, 1], F32, kind="ExternalInput").ap()
    wdcl = nc.dram_tensor("wdcl", [P, K2, 2, 2, P], BF, kind="ExternalInput").ap()
    bdc = nc.dram_tensor("bdc", [P, 2], F32, kind="ExternalInput").ap()
    pyb = nc.dram_tensor("pyb", [P, NTILE, K2], F32, kind="ExternalInput").ap()
    pxb = nc.dram_tensor("pxb", [P, NTILE, K2], F32, kind="ExternalInput").ap()
    rep = nc.dram_tensor("rep", [P, 8, P], F32, kind="ExternalInput").ap()
    out = nc.dram_tensor("out", [2, P, NPIX], F32, kind="ExternalOutput").ap()
    with tile.TileContext(nc) as tc:
        _emit(tc, nc, (pt, xc, wofl, boff, wdcl, bdc, pyb, pxb, rep, out))
    nc.compile()
    _BUILT["nc"] = nc
    return nc


def _prep_core(k, x, w_off, b_off, w_dc, b_dc):
    import ml_dtypes

    bf16 = ml_dtypes.bfloat16
    b, half = k // 2, k % 2
    y0 = half * ROWS
    xs = x[b]  # [C,H,W] f32
    xs_t = xs.transpose(1, 2, 0)  # [H, W, C]
    # row-pair token table: token (j, x) = [row j-1, row j], zero padded
    pt = np.zeros((NPTOK, 2 * C), np.float32)
    view = pt[: 97 * W].reshape(97, W, 2 * C)
    view[1:, :, 0:C] = xs_t
    view[:H, :, C : 2 * C] = xs_t

    xc = np.zeros((C, PADH, PADW), np.float32)
    r0, r1 = max(0, y0 - 1), min(H, y0 + ROWS + 1)
    xc[:, (r0 - (y0 - 1)) : (r1 - (y0 - 1)), 1 : 1 + W] = xs[:, r0:r1, :]
    xc = xc.reshape(2, P, PADH * PADW).transpose(1, 0, 2)

    wofl = (
        w_off.reshape(2 * K2, 2, P, K2)   # [oc, chalf, c, tap]
        .transpose(2, 1, 3, 0)            # [c, chalf, tap, oc]
        .copy()
    )
    wdcl = (
        w_dc.reshape(2, P, 2, P, K2)      # [oh, o, chalf, c, tap]
        .transpose(3, 4, 2, 0, 1)         # [c, tap, chalf, oh, o]
        .copy()
    )
    bdc = b_dc.reshape(2, P).transpose(1, 0).copy()

    pp = np.arange(NPIX)
    yg = y0 + pp // W
    xg = pp % W
    ti = (np.arange(K2) // K)[None, :]
    tj = (np.arange(K2) % K)[None, :]
    # +16 shift baked in (device math keeps coords strictly positive)
    pyb = (yg[:, None] - 1 + ti + 16).astype(np.float32).reshape(NTILE, P, K2)
    pxb = (xg[:, None] - 1 + tj + 16).astype(np.float32).reshape(NTILE, P, K2)

    # replicated identity slices: rep[16k+r, k, r::16] = 1
    repm = np.zeros((P, 8, P), np.float32)
    for kk in range(8):
        for r in range(16):
            repm[16 * kk + r, kk, r::16] = 1.0

    return {
        "pt": pt.astype(bf16),
        "rep": repm,
        "xc": xc.astype(bf16),
        "wofl": wofl.astype(bf16),
        "boff": b_off.reshape(18, 1).astype(np.float32),
        "wdcl": wdcl.astype(bf16),
        "bdc": bdc.astype(np.float32),
        "pyb": pyb.transpose(1, 0, 2).copy(),
        "pxb": pxb.transpose(1, 0, 2).copy(),
    }


def kernel(x, w_off, b_off, w_dc, b_dc, _trace=False):
    nc = _build()
    x = np.asarray(x, np.float32)
    w_off = np.asarray(w_off, np.float32)
    b_off = np.asarray(b_off, np.float32)
    w_dc = np.asarray(w_dc, np.float32)
    b_dc = np.asarray(b_dc, np.float32)
    in_maps = [
        _prep_core(k, x, w_off, b_off, w_dc, b_dc) for k in range(NCORES)
    ]
    res = bass_utils.run_bass_kernel_spmd(
        nc, in_maps, core_ids=list(range(NCORES)), trace=_trace
    )
    out = np.empty((B, O, H, W), np.float32)
    for k in range(NCORES):
        b, half = k // 2, k % 2
        o = res.results[k]["out"]  # [2,128,4608]
        out[b, :, half * ROWS : (half + 1) * ROWS, :] = o.reshape(
            O, ROWS, W
        )
    if _trace:
        return out, res
    return out


# revision 14
# speedup vs baseline: 1.0925x; 1.0194x over previous
"""Deformable conv (3x3, with offset-predicting conv) for Trainium2, 8 cores.

Sharding: pure data parallel. Core k handles sample b = k//2, output row block
(k%2)*48 .. +48 (48 rows x 96 cols = 4608 pixels).

v2 pipeline per core:
  A. offset conv (3x3, C=256 -> 18) as 18 PE matmuls per 4-row chunk
  B. PE-transpose offsets into pixel-partition layout doff [128px, 36t, 18]
  C. (per 3-stage group) DVE weight chain -> factored bilinear weights
     wa/wb/wy0/wy1 [128, 36, 9] f32, plus gather index idxf (f32, exact ints)
  D. (per group) fold idxf into the SWDGE wrapped-16-partition layout via 8
     identity-slice PE matmuls + strided int16 copies; replicate x8 with 3
     log-doubling DMAs (replaces v1's 129 tiny strided DMAs ~800us)
  E. per (stage of 512px, tap): ONE dma_gather on a host-built row-pair token
     table (2KB descriptor = all 4 bilinear corners of a pixel); factored
     combine split across DVE/ACT/Pool; ONE XBAR dma transpose per tap to
     channel layout (replaces 8 PE transposes + 8 PSUM copies); PE matmul
     accumulating over (c, tap) into PSUM [o, px]; bias + store.
"""

import dataclasses

import numpy as np

import concourse.bacc as bacc
import concourse.bass as bass
import concourse.mybir as mybir
import concourse.tile as tile
from concourse import bass_utils, masks
from concourse.mybir import ActivationFunctionType as Act
from concourse.mybir import AluOpType as Op

P = 128
B, C, H, W, O = 4, 256, 96, 96, 256
K = 3
K2 = 9
NCORES = 8
ROWS = 48                      # output rows per core
NPIX = ROWS * W                # 4608
NTILE = NPIX // P              # 36 pixel tiles of 128
NSTAGE = 9                     # stages of 512 px
TPS = 4                        # pixel tiles per stage
SPX = TPS * P                  # 512
NGRP = 3                       # stage groups for prefix/stage-E overlap
SPG = NSTAGE // NGRP           # stages per group = 3
TPG = SPG * TPS                # tiles per group = 12
PADH, PADW = ROWS + 2, W + 2   # 50, 98
NPTOK = 97 * W + 2             # row-pair tokens (j in 0..96) + 2 zero pads
CONV_ROWS_PER_CHUNK = 4        # offset-conv N chunk = 4 rows = 384 cols
NCHUNK = ROWS // CONV_ROWS_PER_CHUNK  # 12
BF = mybir.dt.bfloat16
F32 = mybir.dt.float32
I16 = mybir.dt.int16

_BUILT = {}


def _emit(tc, nc, io):
    pt, xc, wofl, boff, wdcl, bdc, pyb, pxb, rep, out = io

    with (
        tc.tile_pool(name="const", bufs=1) as cpool,
        tc.tile_pool(name="sbig", bufs=1) as spool,
    ):
        ident_f = cpool.tile([P, P], F32, tag="idf", name="idf")
        masks.make_identity(nc, ident_f[:])

        # ---- persistent SBUF buffers ----
        xc_sb = spool.tile([P, 2, PADH * PADW], BF, tag="xc", name="xc")
        wofl_sb = spool.tile([P, 2, K2, 18], BF, tag="wofl", name="wofl")
        wdcl_sb = spool.tile([P, K2, 2, 2, P], BF, tag="wdcl", name="wdcl")
        boff_sb = spool.tile([18, 1], F32, tag="boff", name="boff")
        bdc_sb = spool.tile([P, 2], F32, tag="bdc", name="bdc")
        pyb_sb = spool.tile([P, NTILE, K2], F32, tag="pyb", name="pyb")
        pxb_sb = spool.tile([P, NTILE, K2], F32, tag="pxb", name="pxb")
        rep_sb = spool.tile([P, 8, P], F32, tag="rep", name="rep")
        off_sb = spool.tile([18, NPIX], F32, tag="off", name="off")
        doff = spool.tile([P, NTILE, 18], F32, tag="doff", name="doff")
        # factored bilinear weights (per px, tap)
        wa_sb = spool.tile([P, NTILE, K2], F32, tag="wa", name="wa")
        wb_sb = spool.tile([P, NTILE, K2], F32, tag="wb", name="wb")
        wy0_sb = spool.tile([P, NTILE, K2], F32, tag="wy0", name="wy0")
        wy1_sb = spool.tile([P, NTILE, K2], F32, tag="wy1", name="wy1")
        # wrapped gather indices [16 parts x8 rep, (s, tap, t, k)]
        cidxw = spool.tile([P, NSTAGE, K2, TPS, 8], I16, tag="cidxw", name="cidxw")

        nc.sync.dma_start(xc_sb[:], xc)
        nc.sync.dma_start(wofl_sb[:], wofl)
        nc.sync.dma_start(wdcl_sb[:], wdcl)
        nc.sync.dma_start(boff_sb[:], boff)
        nc.sync.dma_start(bdc_sb[:], bdc)
        nc.sync.dma_start(pyb_sb[:], pyb)
        nc.sync.dma_start(pxb_sb[:], pxb)
        nc.sync.dma_start(rep_sb[:], rep)

        with (
            tc.tile_pool(name="psA", bufs=2, space="PSUM") as psa,
            tc.tile_pool(name="psB", bufs=1, space="PSUM") as psb,
            tc.tile_pool(name="psF", bufs=1, space="PSUM") as psf_pool,
            tc.tile_pool(name="scr", bufs=2) as scr,
            tc.tile_pool(name="gpool", bufs=6) as gpool,
            tc.tile_pool(name="mpool", bufs=8) as mpool,
            tc.tile_pool(name="vapool", bufs=6) as vapool,
            tc.tile_pool(name="rpool", bufs=6) as rpool,
            tc.tile_pool(name="opool", bufs=4) as opool,
            tc.tile_pool(name="psO", bufs=2, space="PSUM") as pso,
        ):
            # ---- A: offset conv ----
            for ch_i in range(NCHUNK):
                ncols = CONV_ROWS_PER_CHUNK * W  # 384
                ps = psa.tile([18, ncols], F32, tag="psoff", name="psoff")
                n_mm = 2 * K2
                mm = 0
                xcf = xc_sb[:]
                for chalf in range(2):
                    for tap in range(K2):
                        ti, tj = tap // K, tap % K
                        rhs = dataclasses.replace(
                            xcf,
                            ap=[
                                [xcf.ap[0][0], P],
                                [PADW, CONV_ROWS_PER_CHUNK],
                                [1, W],
                            ],
                            offset=xcf.offset
                            + chalf * (PADH * PADW)
                            + ((ch_i * CONV_ROWS_PER_CHUNK + ti) * PADW + tj),
                        )
                        nc.tensor.matmul(
                            ps[:],
                            wofl_sb[:, chalf, tap],
                            rhs,
                            start=(mm == 0),
                            stop=(mm == n_mm - 1),
                        )
                        mm += 1
                nc.scalar.activation(
                    off_sb[:, ch_i * ncols : (ch_i + 1) * ncols],
                    ps[:],
                    Act.Identity,
                    bias=boff_sb[:],
                )

            # ---- B: transpose offsets to pixel layout ----
            for t in range(NTILE):
                pt_ps = psb.tile([P, 18], F32, tag="pofft", name="pofft")
                nc.tensor.transpose(
                    pt_ps[:], off_sb[:, t * P : (t + 1) * P], ident_f[:18, :18]
                )
                nc.scalar.copy(doff[:, t, :], pt_ps[:])

            # ---- C+D per group, then E per stage ----
            MAGIC = 8388608.0  # 2^23

            def chain(g):
                # weight + index math on [P, TPG, K2] slices (group g)
                sl = slice(g * TPG, (g + 1) * TPG)
                sh = [P, TPG, K2]

                def tmp(tag):
                    return scr.tile(sh, F32, tag=tag, name=tag)

                dy = doff[:, sl, 0:18:2]
                dx = doff[:, sl, 1:18:2]
                # pyb/pxb already carry +16 (host); coords strictly positive
                py = tmp("py")
                px = tmp("px")
                nc.vector.tensor_tensor(py[:], pyb_sb[:, sl], dy, Op.add)
                nc.vector.tensor_tensor(px[:], pxb_sb[:, sl], dx, Op.add)
                y0 = tmp("y0")
                x0 = tmp("x0")
                nc.vector.tensor_scalar(y0[:], py[:], -0.4999999, None, Op.add)
                nc.vector.tensor_scalar(y0[:], y0[:], MAGIC, -MAGIC, Op.add, Op.add)
                nc.vector.tensor_scalar(x0[:], px[:], -0.4999999, None, Op.add)
                nc.vector.tensor_scalar(x0[:], x0[:], MAGIC, -MAGIC, Op.add, Op.add)
                ly = tmp("ly")
                lx = tmp("lx")
                nc.vector.tensor_tensor(ly[:], py[:], y0[:], Op.subtract)
                nc.vector.tensor_tensor(lx[:], px[:], x0[:], Op.subtract)

                ta_ = tmp("ta")
                tb_ = tmp("tb")
                tc_ = tmp("tc")
                # y: wy0 = (1-ly)*[16<=y0<=111], wy1 = ly*[15<=y0<=110]
                nc.vector.tensor_scalar(ta_[:], y0[:], 16.0, None, Op.is_ge)
                nc.vector.tensor_scalar(tb_[:], y0[:], 111.0, None, Op.is_le)
                vy0 = tmp("vy0")
                nc.vector.tensor_tensor(vy0[:], ta_[:], tb_[:], Op.mult)
                nc.vector.tensor_scalar(ta_[:], y0[:], 15.0, None, Op.is_ge)
                nc.vector.tensor_scalar(tb_[:], y0[:], 110.0, None, Op.is_le)
                vy1 = tmp("vy1")
                nc.vector.tensor_tensor(vy1[:], ta_[:], tb_[:], Op.mult)
                nc.vector.tensor_scalar(tc_[:], ly[:], -1.0, 1.0, Op.mult, Op.add)
                nc.vector.tensor_tensor(wy0_sb[:, sl], tc_[:], vy0[:], Op.mult)
                nc.vector.tensor_tensor(wy1_sb[:, sl], ly[:], vy1[:], Op.mult)

                # x pair weights: wa = (1-lx)*[16<=x0<=111] + lx*[x0==15]
                #                 wb = lx*[16<=x0<=110]
                nc.vector.tensor_scalar(ta_[:], x0[:], 16.0, None, Op.is_ge)
                nc.vector.tensor_scalar(tb_[:], x0[:], 111.0, None, Op.is_le)
                vx = tmp("vx")
                nc.vector.tensor_tensor(vx[:], ta_[:], tb_[:], Op.mult)
                nc.vector.tensor_scalar(tb_[:], x0[:], 110.0, None, Op.is_le)
                vxb = tmp("vxb")
                nc.vector.tensor_tensor(vxb[:], ta_[:], tb_[:], Op.mult)
                td_ = tmp("td")
                nc.vector.tensor_scalar(td_[:], x0[:], 15.0, None, Op.is_equal)
                nc.vector.tensor_scalar(tc_[:], lx[:], -1.0, 1.0, Op.mult, Op.add)
                nc.vector.tensor_tensor(tc_[:], tc_[:], vx[:], Op.mult)
                nc.vector.tensor_tensor(td_[:], lx[:], td_[:], Op.mult)
                nc.vector.tensor_tensor(wa_sb[:, sl], tc_[:], td_[:], Op.add)
                nc.vector.tensor_tensor(wb_sb[:, sl], lx[:], vxb[:], Op.mult)

                # gather index (f32, exact ints): pair-token j=clip(y0r,-1,95)+1,
                # xb=clip(x0r,0,95); idx = j*96+xb. Shifted: jsh=clip(y0,15,111),
                # xbsh=clip(x0,16,111), idx = jsh*96 + xbsh - 1456.
                jsh = tmp("jsh")
                xbsh = tmp("xbsh")
                nc.vector.tensor_scalar(jsh[:], y0[:], 15.0, 111.0, Op.max, Op.min)
                nc.vector.tensor_scalar(xbsh[:], x0[:], 16.0, 111.0, Op.max, Op.min)
                idxf = tmp("idxf")
                nc.vector.scalar_tensor_tensor(
                    idxf[:], jsh[:], 96.0, xbsh[:], Op.mult, Op.add
                )
                nc.vector.tensor_scalar(idxf[:], idxf[:], -1456.0, None, Op.add)

                # fold to wrapped layout: value for partition 16k+r lands on
                # ALL partitions 16g+r (rep_sb[:, k] = replicated ident slice),
                # so no replication DMAs are needed on the index path.
                for k in range(8):
                    psf = psf_pool.tile([P, TPG * K2], F32, tag="psf", name="psf")
                    nc.tensor.matmul(
                        psf[:],
                        rep_sb[:, k],
                        idxf[:],
                        start=True,
                        stop=True,
                    )
                    # src iter (s_loc, t_loc, tap) -> dst strides (288, 8, 32)
                    cw = cidxw[:]
                    dst = dataclasses.replace(
                        cw,
                        ap=[
                            [cw.ap[0][0], P],
                            [K2 * TPS * 8, SPG],
                            [8, TPS],
                            [TPS * 8, K2],
                        ],
                        offset=cw.offset + g * SPG * K2 * TPS * 8 + k,
                    )
                    src = dataclasses.replace(
                        psf[:],
                        ap=[
                            [psf[:].ap[0][0], P],
                            [TPS * K2, SPG],
                            [K2, TPS],
                            [1, K2],
                        ],
                    )
                    nc.vector.tensor_copy(dst, src)

            # overlapped-window view of pair-token table: [tok, 1024] stride 512
            pt_win = dataclasses.replace(
                pt, ap=[[2 * C, NPTOK - 1], [1, 4 * C]], offset=0
            )

            def stage(s):
                po = [
                    pso.tile([P, SPX], F32, tag=f"po{oh}", name=f"po{oh}")
                    for oh in range(2)
                ]
                for tap in range(K2):
                    gt = gpool.tile([P, TPS, 4 * C], BF, tag="g", name="g")
                    for hf in range(2):
                        nc.gpsimd.dma_gather(
                            gt[:, 2 * hf : 2 * (hf + 1), :],
                            pt_win,
                            cidxw[:, s, tap, 2 * hf : 2 * (hf + 1)],
                            SPX // 2,
                            SPX // 2,
                            elem_size=4 * C,
                            elem_step=2 * C,
                        )
                    va = vapool.tile([P, TPS, 2 * P], BF, tag="va", name="va")
                    for t in range(TPS):
                        st_g = s * TPS + t
                        wa_c = wa_sb[:, st_g, tap : tap + 1]
                        wb_c = wb_sb[:, st_g, tap : tap + 1]
                        wy0_c = wy0_sb[:, st_g, tap : tap + 1]
                        wy1_c = wy1_sb[:, st_g, tap : tap + 1]
                        m_ = mpool.tile([P, 2 * C], BF, tag="m", name="m")
                        s_ = mpool.tile([P, 2 * C], BF, tag="s", name="s")
                        h0 = mpool.tile([P, C], BF, tag="h0", name="h0")
                        h1 = mpool.tile([P, C], BF, tag="h1", name="h1")
                        nc.vector.tensor_scalar(
                            m_[:], gt[:, t, 0 : 2 * C], wa_c, None, Op.mult
                        )
                        nc.vector.scalar_tensor_tensor(
                            s_[:], gt[:, t, 2 * C : 4 * C], wb_c, m_[:],
                            Op.mult, Op.add,
                        )
                        nc.scalar.activation(
                            h0[:], s_[:, 0:C], Act.Identity, scale=wy0_c
                        )
                        nc.scalar.activation(
                            h1[:], s_[:, C : 2 * C], Act.Identity, scale=wy1_c
                        )
                        nc.vector.tensor_tensor(
                            va[:, t, :], h0[:], h1[:], Op.add
                        )
                    # XBAR transpose: [128px, (t,chalf,c)] -> [128c, (t,chalf), px]
                    rst = rpool.tile([P, 2 * TPS, P], BF, tag="rst", name="rst")
                    nc.sync.dma_start_transpose(rst[:], va[:])
                    for chalf in range(2):
                        rv = rst[:]
                        rhs = dataclasses.replace(
                            rv,
                            ap=[[rv.ap[0][0], P], [2 * P, TPS], [1, P]],
                            offset=rv.offset + chalf * P,
                        )
                        for oh in range(2):
                            nc.tensor.matmul(
                                po[oh][:],
                                wdcl_sb[:, tap, chalf, oh],
                                rhs,
                                start=(tap == 0 and chalf == 0),
                                stop=(tap == K2 - 1 and chalf == 1),
                            )
                for oh in range(2):
                    ob = opool.tile([P, SPX], F32, tag="ob", name="ob")
                    nc.scalar.activation(
                        ob[:], po[oh][:], Act.Identity, bias=bdc_sb[:, oh : oh + 1]
                    )
                    nc.sync.dma_start(out[oh, :, s * SPX : (s + 1) * SPX], ob[:])

            for g in range(NGRP):
                chain(g)
                for s in range(g * SPG, (g + 1) * SPG):
                    stage(s)


def _build():
    if "nc" in _BUILT:
        return _BUILT["nc"]
    nc = bacc.Bacc(
        "TRN2",
        target_bir_lowering=False,
        debug=False,
        enable_asserts=False,
        num_devices=NCORES,
    )
    pt = nc.dram_tensor("pt", [NPTOK, 2 * C], BF, kind="ExternalInput").ap()
    xc = nc.dram_tensor("xc", [P, 2, PADH * PADW], BF, kind="ExternalInput").ap()
    wofl = nc.dram_tensor("wofl", [P, 2, K2, 18], BF, kind="ExternalInput").ap()
    boff = nc.dram_tensor("boff", [18, 1], F32, kind="ExternalInput").ap()
    wdcl = nc.dram_tensor("wdcl", [P, K2, 2, 2, P], BF, kind="ExternalInput").ap()
    bdc = nc.dram_tensor("bdc", [P, 2], F32, kind="ExternalInput").ap()
    pyb = nc.dram_tensor("pyb", [P, NTILE, K2], F32, kind="ExternalInput").ap()
    pxb = nc.dram_tensor("pxb", [P, NTILE, K2], F32, kind="ExternalInput").ap()
    rep = nc.dram_tensor("rep", [P, 8, P], F32, kind="ExternalInput").ap()
    out = nc.dram_tensor("out", [2, P, NPIX], F32, kind="ExternalOutput").ap()
    with tile.TileContext(nc) as tc:
        _emit(tc, nc, (pt, xc, wofl, boff, wdcl, bdc, pyb, pxb, rep, out))
    nc.compile()
    _BUILT["nc"] = nc
    return nc


def _prep_core(k, x, w_off, b_off, w_dc, b_dc):
    import ml_dtypes

    bf16 = ml_dtypes.bfloat16
    b, half = k // 2, k % 2
    y0 = half * ROWS
    xs = x[b]  # [C,H,W] f32
    xs_t = xs.transpose(1, 2, 0)  # [H, W, C]
    # row-pair token table: token (j, x) = [row j-1, row j], zero padded
    pt = np.zeros((NPTOK, 2 * C), np.float32)
    view = pt[: 97 * W].reshape(97, W, 2 * C)
    view[1:, :, 0:C] = xs_t
    view[:H, :, C : 2 * C] = xs_t

    xc = np.zeros((C, PADH, PADW), np.float32)
    r0, r1 = max(0, y0 - 1), min(H, y0 + ROWS + 1)
    xc[:, (r0 - (y0 - 1)) : (r1 - (y0 - 1)), 1 : 1 + W] = xs[:, r0:r1, :]
    xc = xc.reshape(2, P, PADH * PADW).transpose(1, 0, 2)

    wofl = (
        w_off.reshape(2 * K2, 2, P, K2)   # [oc, chalf, c, tap]
        .transpose(2, 1, 3, 0)            # [c, chalf, tap, oc]
        .copy()
    )
    wdcl = (
        w_dc.reshape(2, P, 2, P, K2)      # [oh, o, chalf, c, tap]
        .transpose(3, 4, 2, 0, 1)         # [c, tap, chalf, oh, o]
        .copy()
    )
    bdc = b_dc.reshape(2, P).transpose(1, 0).copy()

    pp = np.arange(NPIX)
    yg = y0 + pp // W
    xg = pp % W
    ti = (np.arange(K2) // K)[None, :]
    tj = (np.arange(K2) % K)[None, :]
    # +16 shift baked in (device math keeps coords strictly positive)
    pyb = (yg[:, None] - 1 + ti + 16).astype(np.float32).reshape(NTILE, P, K2)
    pxb = (xg[:, None] - 1 + tj + 16).astype(np.float32).reshape(NTILE, P, K2)

    # replicated identity slices: rep[16k+r, k, r::16] = 1
    repm = np.zeros((P, 8, P), np.float32)
    for kk in range(8):
        for r in range(16):
            repm[16 * kk + r, kk, r::16] = 1.0

    return {
        "pt": pt.astype(bf16),
        "rep": repm,
        "xc": xc.astype(bf16),
        "wofl": wofl.astype(bf16),
        "boff": b_off.reshape(18, 1).astype(np.float32),
        "wdcl": wdcl.astype(bf16),
        "bdc": bdc.astype(np.float32),
        "pyb": pyb.transpose(1, 0, 2).copy(),
        "pxb": pxb.transpose(1, 0, 2).copy(),
    }


def kernel(x, w_off, b_off, w_dc, b_dc, _trace=False):
    nc = _build()
    x = np.asarray(x, np.float32)
    w_off = np.asarray(w_off, np.float32)
    b_off = np.asarray(b_off, np.float32)
    w_dc = np.asarray(w_dc, np.float32)
    b_dc = np.asarray(b_dc, np.float32)
    in_maps = [
        _prep_core(k, x, w_off, b_off, w_dc, b_dc) for k in range(NCORES)
    ]
    res = bass_utils.run_bass_kernel_spmd(
        nc, in_maps, core_ids=list(range(NCORES)), trace=_trace
    )
    out = np.empty((B, O, H, W), np.float32)
    for k in range(NCORES):
        b, half = k // 2, k % 2
        o = res.results[k]["out"]  # [2,128,4608]
        out[b, :, half * ROWS : (half + 1) * ROWS, :] = o.reshape(
            O, ROWS, W
        )
    if _trace:
        return out, res
    return out
